# revision 26
# baseline (speedup 1.0000x reference)
"""Trainium2 Bass kernel for nn_Block_7584912244938.

Dense transformer block: self-attn + img cross-attn + prompt cross-attn + MLP,
d_model=1024, 16 heads x 64, batch 256 x 60 tokens.

Strategy: pure data parallel over batch across 8 NeuronCores (32 items/core).
Inside each core: token-major activations with PE transposes feeding
feature-major (transposed) matmul operands; bf16 matmuls with fp32 PSUM
accumulation; LN/softmax/residual arithmetic in fp32.

Self-contained: only imports from /opt/trn_rl_repo (baked into the container).
"""

import contextlib
import sys

sys.path.insert(0, "/opt/trn_rl_repo")

import ml_dtypes
import numpy as np
import orjson

import concourse.bass as bass
import concourse.tile as tile
from concourse import mybir

F32 = mybir.dt.float32
BF16 = mybir.dt.bfloat16
AX = mybir.AxisListType.X
MUL = mybir.AluOpType.mult
ADD = mybir.AluOpType.add
SUB = mybir.AluOpType.subtract
AF = mybir.ActivationFunctionType

# problem dims (per core)
NCORES = 8
B = 32            # items per core
T = 60            # x tokens per item
NTOK = B * T      # 1920
D = 1024
H = 16
DH = 64
INNER = 1024
VIS = 768
SI = 197          # img tokens per item
SP = 77           # prompt tokens per item
SCALE = DH ** -0.5
NT = NTOK // 128  # 15 token tiles of 128
KC_D = D // 128   # 8
KC_V = VIS // 128  # 6
EPS = 1e-5


def _fix_bir_waits(d):
    """This container's walrus accepts at most 1 sync-wait per instruction;
    Tile emits up to ~3. Split the excess onto inserted single-wait NoOps
    placed immediately before (same engine, so ordering is preserved)."""
    for fn in d["functions"]:
        for blk in fn["blocks"]:
            new_insts = []
            for ins in blk["instructions"]:
                si = ins.get("sync_info") or {}
                w = si.get("on_wait") or []
                cap = 1
                if len(w) > cap:
                    for j, ww in enumerate(w[:-cap]):
                        new_insts.append({
                            "engine": ins["engine"], "ins": [], "outs": [],
                            "is_reset_sema": False,
                            "name": f"{ins['name']}__ws{j}",
                            "opcode": "NoOp",
                            "sync_info": {"on_update": [], "on_wait": [ww]},
                        })
                    si = dict(si)
                    si["on_wait"] = w[-cap:]
                    ins = dict(ins)
                    ins["sync_info"] = si
                new_insts.append(ins)
            blk["instructions"] = new_insts


def patch_nc_for_walrus(nc):
    orig = nc.to_json_bytes

    def patched():
        d = orjson.loads(orig())
        _fix_bir_waits(d)
        return orjson.dumps(d)

    nc.to_json_bytes = patched


def build_nc(n_phases=4, stop_after=None):
    nc = bass.Bass("TRN2", target_bir_lowering=False, debug=False)

    x_in = nc.dram_tensor("x", [NTOK, D], F32, kind="ExternalInput")
    img_in = nc.dram_tensor("img_embd", [B, SI, VIS], F32, kind="ExternalInput")
    pr_in = nc.dram_tensor("prompt_embd", [B, SP, VIS], F32,
                           kind="ExternalInput")

    wdram = {}
    for p in ("sa", "ia", "pa"):
        ckin = D if p == "sa" else VIS
        wdram[p + "_wq"] = nc.dram_tensor(p + "_wq", [D, INNER], F32,
                                          kind="ExternalInput")
        wdram[p + "_wk"] = nc.dram_tensor(p + "_wk", [ckin, INNER], F32,
                                          kind="ExternalInput")
        wdram[p + "_wv"] = nc.dram_tensor(p + "_wv", [ckin, INNER], F32,
                                          kind="ExternalInput")
        wdram[p + "_wo"] = nc.dram_tensor(p + "_wo", [INNER, D], F32,
                                          kind="ExternalInput")
        wdram[p + "_bo"] = nc.dram_tensor(p + "_bo", [1, D], F32,
                                          kind="ExternalInput")
    for i in (1, 2, 3, 4):
        wdram[f"ln{i}_w"] = nc.dram_tensor(f"ln{i}_w", [1, D], F32,
                                           kind="ExternalInput")
        wdram[f"ln{i}_b"] = nc.dram_tensor(f"ln{i}_b", [1, D], F32,
                                           kind="ExternalInput")
    wdram["fc_w"] = nc.dram_tensor("fc_w", [D, 4 * D], F32,
                                   kind="ExternalInput")
    wdram["fc_b"] = nc.dram_tensor("fc_b", [32, 128], F32,
                                   kind="ExternalInput")
    wdram["proj_w"] = nc.dram_tensor("proj_w", [4 * D, D], F32,
                                     kind="ExternalInput")
    wdram["proj_b"] = nc.dram_tensor("proj_b", [1, D], F32,
                                     kind="ExternalInput")

    out_d = nc.dram_tensor("out", [NTOK, D], F32, kind="ExternalOutput")
    xs = [nc.dram_tensor(f"x_scr{i}", [NTOK, D], F32) for i in range(3)]
    ident_bd = nc.inline_tensor(np.eye(128).astype(ml_dtypes.bfloat16),
                                name="ident_bf")
    ones_d = nc.inline_tensor(np.ones((1, 128), np.float32), name="ones_f32")

    with tile.TileContext(nc) as tc, contextlib.ExitStack() as top:
        const = top.enter_context(tc.tile_pool(name="const", bufs=1))
        ident = const.tile([128, 128], BF16, tag="ident")
        nc.sync.dma_start(ident[:], ident_bd[:])
        ones_f = const.tile([1, 128], F32, tag="ones")
        nc.sync.dma_start(ones_f[:], ones_d[:])

        def bcast_load(pool, dram_h, cols, tag):
            # broadcast [1, cols] across partitions via a K=1 matmul with a
            # ones column (this walrus rejects stride-0 partition DMA APs)
            row = pool.tile([1, cols], F32, tag=tag + "r")
            nc.sync.dma_start(row[:], dram_h[:])
            t = pool.tile([128, cols], F32, tag=tag)
            with tc.tile_pool(name="bc_" + tag, bufs=2, space="PSUM") as bp:
                for j in range(cols // 512):
                    ps = bp.tile([128, 512], F32, tag="b")
                    nc.tensor.matmul(ps[:], ones_f[:, :],
                                     row[:, 512 * j:512 * (j + 1)],
                                     start=True, stop=True)
                    nc.vector.tensor_copy(t[:, 512 * j:512 * (j + 1)], ps[:])
            return t

        def load_weight_bf16(dst_pool, dram_h, kin, kout, tag):
            """[kin,kout] f32 dram -> [128, kin//128, kout] bf16 sbuf."""
            kc = kin // 128
            with tc.tile_pool(name="wstage_" + tag, bufs=1) as sp:
                wf = sp.tile([128, kc, kout], F32, tag="wstage")
                nc.sync.dma_start(
                    wf[:], dram_h[:].rearrange("(c p) o -> p c o", p=128))
                wb = dst_pool.tile([128, kc, kout], BF16, tag=tag)
                nc.vector.tensor_copy(wb[:], wf[:])
            return wb

        def ln_transpose_stage(src_dram, lnw_t, lnb_t, hT):
            """LayerNorm all 15 x-tiles of src; write transposed bf16 into
            hT [128, 8, 1920]."""
            with contextlib.ExitStack() as st:
                xp = st.enter_context(tc.tile_pool(name="ln_x", bufs=2))
                stat = st.enter_context(tc.tile_pool(name="ln_stat", bufs=4))
                tp = st.enter_context(
                    tc.tile_pool(name="ln_tr", bufs=2, space="PSUM"))
                for t_i in range(NT):
                    xt = xp.tile([128, D], F32, tag="x")
                    nc.sync.dma_start(
                        xt[:], src_dram[128 * t_i:128 * (t_i + 1), :])
                    s1 = stat.tile([128, 1], F32, tag="s1")
                    nc.vector.reduce_sum(s1[:], xt[:], axis=AX)
                    sq = xp.tile([128, D], F32, tag="scr")
                    nc.vector.tensor_mul(sq[:], xt[:], xt[:])
                    ssq = stat.tile([128, 1], F32, tag="ssq")
                    nc.vector.reduce_sum(ssq[:], sq[:], axis=AX)
                    mu = stat.tile([128, 1], F32, tag="mu")
                    nc.vector.tensor_scalar_mul(mu[:], s1[:], 1.0 / D)
                    mu2 = stat.tile([128, 1], F32, tag="mu2")
                    nc.vector.tensor_mul(mu2[:], mu[:], mu[:])
                    var = stat.tile([128, 1], F32, tag="var")
                    nc.vector.tensor_scalar_mul(var[:], ssq[:], 1.0 / D)
                    nc.vector.tensor_sub(var[:], var[:], mu2[:])
                    nc.vector.tensor_scalar_add(var[:], var[:], EPS)
                    # this walrus requires ACT ops to be [128, >=128]: run the
                    # sqrt on a zero-padded [128, 128] tile
                    varp = stat.tile([128, 128], F32, tag="varp")
                    nc.vector.memset(varp[:], 1.0)
                    nc.vector.tensor_copy(varp[:, 0:1], var[:])
                    sdp = stat.tile([128, 128], F32, tag="sdp")
                    nc.scalar.activation(sdp[:], varp[:], AF.Sqrt)
                    rstd = stat.tile([128, 1], F32, tag="rstd")
                    nc.vector.reciprocal(rstd[:], sdp[:, 0:1])
                    xn = xp.tile([128, D], F32, tag="scr")
                    nc.vector.tensor_scalar(
                        out=xn[:], in0=xt[:], scalar1=mu[:], scalar2=rstd[:],
                        op0=SUB, op1=MUL)
                    nc.vector.tensor_mul(xn[:], xn[:], lnw_t[:])
                    hb = xp.tile([128, D], BF16, tag="hb")
                    nc.vector.tensor_add(hb[:], xn[:], lnb_t[:])
                    ps = tp.tile([128, KC_D, 128], BF16, tag="tr")
                    for c in range(KC_D):
                        nc.tensor.transpose(
                            ps[:, c, :], hb[:, 128 * c:128 * (c + 1)],
                            ident[:])
                    nc.vector.tensor_copy(
                        hT[:, :, 128 * t_i:128 * (t_i + 1)], ps[:])

        def project_T(w_bf, src_T, kc_n, dst_T, gsz=480):
            """dst_T [128, 8, NTOK] bf16 (feature-major) = w.T @ src_T."""
            with tc.tile_pool(name="projT", bufs=2, space="PSUM") as pp:
                for g in range(NTOK // gsz):
                    for oc in range(KC_D):
                        ps = pp.tile([128, gsz], F32, tag="p")
                        for kc in range(kc_n):
                            nc.tensor.matmul(
                                ps[:],
                                w_bf[:, kc, 128 * oc:128 * (oc + 1)],
                                src_T[:, kc, gsz * g:gsz * (g + 1)],
                                start=(kc == 0), stop=(kc == kc_n - 1))
                        nc.vector.tensor_copy(
                            dst_T[:, oc, gsz * g:gsz * (g + 1)], ps[:])

        def project_T_pad(w_bf, src_T, kc_n, dst_T, src0, ncols,
                          gsz=480):
            """Head-padded projection: dst_T[0:64, 16, ncols] bf16 =
            per-64-wide-output-slice w.T @ src_T (M=64, base 0)."""
            with tc.tile_pool(name="projP", bufs=2, space="PSUM") as pp:
                for g in range(ncols // gsz):
                    for oc in range(16):
                        ps = pp.tile([64, gsz], F32, tag="p")
                        for kc in range(kc_n):
                            nc.tensor.matmul(
                                ps[:],
                                w_bf[:, kc, 64 * oc:64 * (oc + 1)],
                                src_T[:, kc,
                                      src0 + gsz * g:src0 + gsz * (g + 1)],
                                start=(kc == 0), stop=(kc == kc_n - 1))
                        nc.vector.tensor_copy(
                            dst_T[0:64, oc, gsz * g:gsz * (g + 1)], ps[:])

        def softmax_av(bitem, lb, QT, KTb, S, stiles, v_lhsT, pools,
                       attnT):
            """Scores + exp-softmax + P-transposes + AV for one item.
            QT/KTb are head-padded: [rows 0:64, 16 heads, cols] bf16, so
            every matmul contracts K=64 at partition base 0 (this stack
            hangs on stationary operands at base 64). lb = local col base
            (token col of this item within QT). Scores use M=128 (reads
            zero-padded neighbor cols) so exp sees 128 partitions."""
            simp, pp_pt, pp_av, p_pool, pt_pool, stat = pools
            W = 256 if S > 64 else 64
            P = p_pool.tile([128, H, S], BF16, tag="P")
            for q in range(4):
                sps = simp.tile([128, 4, W], F32, tag="sim")
                for hq in range(4):
                    h = 4 * q + hq
                    nc.tensor.matmul(
                        sps[:, hq, 0:S],
                        QT[0:64, h, lb:lb + 128],
                        KTb[0:64, h, 0:S],
                        start=True, stop=True)
                sc = p_pool.tile([128, 4, W], F32, tag="sc")
                nc.vector.tensor_copy(sc[:, :, 0:S], sps[:, :, 0:S])
                nc.scalar.activation(
                    P[:, 4 * q:4 * (q + 1), :], sc[:, :, 0:S], AF.Exp,
                    scale=SCALE)
            rs = stat.tile([128, H, 1], F32, tag="rs")
            nc.vector.reduce_sum(rs[:, :, 0:1], P[:], axis=AX)
            rsi = stat.tile([128, H, 1], F32, tag="rsi")
            nc.vector.reciprocal(rsi[:], rs[:])
            rsb = stat.tile([128, H, 1], BF16, tag="rsb")
            nc.vector.tensor_copy(rsb[:], rsi[:])
            nc.vector.tensor_tensor(
                out=P[:], in0=P[:], in1=rsb[:].to_broadcast((128, H, S)),
                op=MUL)
            nblk = H * len(stiles)
            PTs = pt_pool.tile([128, nblk, 64], BF16, tag="PTs")
            for si_, (s0, srows) in enumerate(stiles):
                pt_ps = pp_pt.tile([128, H, 64], BF16, tag="pt")
                for h in range(H):
                    nc.tensor.transpose(
                        pt_ps[0:srows, h, 0:60], P[0:60, h, s0:s0 + srows],
                        ident[0:60, 0:60])
                nc.vector.tensor_copy(
                    PTs[0:srows, si_ * H:(si_ + 1) * H, 0:60],
                    pt_ps[0:srows, :, 0:60])
            avp = pp_av.tile([128, KC_D, 64], F32, tag="av")
            for h in range(H):
                rb = (h % 2) * 64
                for si_, (s0, srows) in enumerate(stiles):
                    nc.tensor.matmul(
                        avp[rb:rb + 64, h // 2, 0:60],
                        v_lhsT(si_, h),
                        PTs[0:srows, si_ * H + h, 0:60],
                        start=(si_ == 0), stop=(si_ == len(stiles) - 1))
            nc.vector.tensor_copy(
                attnT[:, :, T * bitem:T * bitem + T], avp[:, :, 0:60])

        def z_residual_stage(attnT, wo_bf, bo_t, xprev_dram, xnext_dram):
            with contextlib.ExitStack() as st:
                zp = st.enter_context(
                    tc.tile_pool(name="zps", bufs=2, space="PSUM"))
                xo = st.enter_context(tc.tile_pool(name="zout", bufs=2))
                for g in range(NT):
                    xt = xo.tile([128, D], F32, tag="xprev")
                    nc.sync.dma_start(
                        xt[:], xprev_dram[128 * g:128 * (g + 1), :])
                    ot = xo.tile([128, D], F32, tag="xout")
                    for nh in range(2):
                        ps = zp.tile([128, 512], F32, tag="z")
                        for kc in range(KC_D):
                            nc.tensor.matmul(
                                ps[:],
                                attnT[:, kc, 128 * g:128 * (g + 1)],
                                wo_bf[:, kc, 512 * nh:512 * (nh + 1)],
                                start=(kc == 0), stop=(kc == KC_D - 1))
                        nc.vector.tensor_add(
                            ot[:, 512 * nh:512 * (nh + 1)], ps[:],
                            xt[:, 512 * nh:512 * (nh + 1)])
                    nc.vector.tensor_add(ot[:], ot[:], bo_t[:])
                    nc.sync.dma_start(
                        xnext_dram[128 * g:128 * (g + 1), :], ot[:])

        # ---------------- phase 1: self attention ----------------
        class _StopBuild(Exception):
            pass

        def _maybe_stop(stage):
            if stop_after == stage:
                raise _StopBuild()

        def phase_self(src_dram, dst_dram):
            with contextlib.ExitStack() as ph:
                pc = ph.enter_context(tc.tile_pool(name="p1const", bufs=1))
                lnw = bcast_load(pc, wdram["ln1_w"], D, "lnw")
                lnb = bcast_load(pc, wdram["ln1_b"], D, "lnb")
                bo = bcast_load(pc, wdram["sa_bo"], D, "bo")
                big = ph.enter_context(tc.tile_pool(name="p1big", bufs=1))
                attnT = big.tile([128, KC_D, NTOK], BF16, tag="attnT")

                with tc.tile_pool(name="p1hT", bufs=1) as hTp:
                    hT = hTp.tile([128, KC_D, NTOK], BF16, tag="hT")
                    _maybe_stop("const")
                    ln_transpose_stage(src_dram, lnw, lnb, hT)
                    _maybe_stop("ln")
                    with tc.tile_pool(name="p1w", bufs=1) as wp, \
                         tc.tile_pool(name="p1qk", bufs=1) as qk:
                        wq = load_weight_bf16(wp, wdram["sa_wq"], D, INNER,
                                              "wq")
                        wk = load_weight_bf16(wp, wdram["sa_wk"], D, INNER,
                                              "wk")
                        wv = load_weight_bf16(wp, wdram["sa_wv"], D, INNER,
                                              "wv")
                        HC = 480 + 128
                        for half in range(4):
                            QTh = qk.tile([64, H, HC], BF16, tag="QTh")
                            for h in range(H):
                                nc.vector.memset(QTh[0:64, h, 480:HC], 0.0)
                            project_T_pad(wq, hT, KC_D, QTh, 480 * half, 480)
                            KTh = qk.tile([64, H, 480], BF16, tag="KTh")
                            project_T_pad(wk, hT, KC_D, KTh, 480 * half, 480)
                            _maybe_stop("qt" if half == 0 else "qt2")
                            with contextlib.ExitStack() as hh:
                                vpool = hh.enter_context(
                                    tc.tile_pool(name="p1V", bufs=1))
                                Vh = vpool.tile([60, 8, INNER], BF16,
                                                tag="V")
                                vps_p = hh.enter_context(tc.tile_pool(
                                    name="p1vps", bufs=2, space="PSUM"))
                                items = list(range(8 * half,
                                                   8 * (half + 1)))
                                for li, bitem in enumerate(items):
                                    for nh in range(2):
                                        vps = vps_p.tile([60, 512], F32,
                                                         tag="vps")
                                        for kc in range(KC_D):
                                            nc.tensor.matmul(
                                                vps[:],
                                                hT[:, kc,
                                                   T * bitem:T * bitem + T],
                                                wv[:, kc,
                                                   512 * nh:512 * (nh + 1)],
                                                start=(kc == 0),
                                                stop=(kc == KC_D - 1))
                                        nc.vector.tensor_copy(
                                            Vh[:, li,
                                               512 * nh:512 * (nh + 1)],
                                            vps[:])
                                _maybe_stop("v" if half == 0 else "v2")
                                simp = hh.enter_context(tc.tile_pool(
                                    name="p1sim", bufs=2, space="PSUM"))
                                pp_pt = hh.enter_context(tc.tile_pool(
                                    name="p1pt", bufs=2, space="PSUM"))
                                pp_av = hh.enter_context(tc.tile_pool(
                                    name="p1av", bufs=2, space="PSUM"))
                                p_pool = hh.enter_context(
                                    tc.tile_pool(name="p1P", bufs=2))
                                pt_pool = hh.enter_context(
                                    tc.tile_pool(name="p1PT", bufs=2))
                                stat = hh.enter_context(
                                    tc.tile_pool(name="p1st", bufs=2))
                                pools = (simp, pp_pt, pp_av, p_pool, pt_pool,
                                         stat)
                                for li, bitem in enumerate(items):
                                    lt = T * li

                                    def v_lhsT(si_, h, _li=li):
                                        return Vh[0:T, _li,
                                                  64 * h:64 * (h + 1)]

                                    softmax_av(
                                        bitem, lt, QTh,
                                        KTh[:, :, lt:lt + T], T, [(0, T)],
                                        v_lhsT, pools, attnT)
                _maybe_stop("attn")
                with tc.tile_pool(name="p1wo", bufs=1) as wop:
                    wo = load_weight_bf16(wop, wdram["sa_wo"], INNER, D,
                                          "wo")
                    z_residual_stage(attnT, wo, bo, src_dram, dst_dram)

        # ---------------- phases 2/3: cross attention ----------------
        def phase_cross(pref, ctx_dram, S, stiles, src_dram, dst_dram, ln_i):
            with contextlib.ExitStack() as ph:
                pc = ph.enter_context(tc.tile_pool(name=pref + "const",
                                                   bufs=1))
                lnw = bcast_load(pc, wdram[f"ln{ln_i}_w"], D, "lnw")
                lnb = bcast_load(pc, wdram[f"ln{ln_i}_b"], D, "lnb")
                bo = bcast_load(pc, wdram[pref + "_bo"], D, "bo")
                big = ph.enter_context(tc.tile_pool(name=pref + "big",
                                                    bufs=1))
                attnT = big.tile([128, KC_D, NTOK], BF16, tag="attnT")
                QT = big.tile([64, H, NTOK + 128], BF16, tag="QT")
                for h in range(H):
                    nc.vector.memset(QT[0:64, h, NTOK:NTOK + 128], 0.0)
                with tc.tile_pool(name=pref + "hT", bufs=1) as hTp:
                    hT = hTp.tile([128, KC_D, NTOK], BF16, tag="hT")
                    ln_transpose_stage(src_dram, lnw, lnb, hT)
                    with tc.tile_pool(name=pref + "wq", bufs=1) as wp:
                        wq = load_weight_bf16(wp, wdram[pref + "_wq"], D,
                                              INNER, "wq")
                        project_T_pad(wq, hT, KC_D, QT, 0, NTOK)
                with contextlib.ExitStack() as il:
                    wkp = il.enter_context(tc.tile_pool(name=pref + "wk",
                                                        bufs=1))
                    wk = load_weight_bf16(wkp, wdram[pref + "_wk"], VIS,
                                          INNER, "wk")
                    wv = load_weight_bf16(wkp, wdram[pref + "_wv"], VIS,
                                          INNER, "wv")
                    cpool = il.enter_context(tc.tile_pool(name=pref + "c",
                                                          bufs=2))
                    ctr = il.enter_context(tc.tile_pool(
                        name=pref + "ctr", bufs=1, space="PSUM"))
                    ktp = il.enter_context(tc.tile_pool(
                        name=pref + "ktp", bufs=1, space="PSUM"))
                    vpp = il.enter_context(tc.tile_pool(
                        name=pref + "vps", bufs=1, space="PSUM"))
                    simp = il.enter_context(tc.tile_pool(
                        name=pref + "sim", bufs=1, space="PSUM"))
                    pp_pt = il.enter_context(tc.tile_pool(
                        name=pref + "pt", bufs=1, space="PSUM"))
                    pp_av = il.enter_context(tc.tile_pool(
                        name=pref + "av", bufs=1, space="PSUM"))
                    kv_pool = il.enter_context(tc.tile_pool(
                        name=pref + "kv", bufs=2))
                    p_pool = il.enter_context(tc.tile_pool(
                        name=pref + "P", bufs=2))
                    pt_pool = il.enter_context(tc.tile_pool(
                        name=pref + "PT", bufs=2))
                    stat = il.enter_context(tc.tile_pool(
                        name=pref + "st", bufs=2))
                    pools = (simp, pp_pt, pp_av, p_pool, pt_pool, stat)
                    for bitem in range(B):
                        ctxT = kv_pool.tile([128, KC_V, S], BF16, tag="ctxT")
                        for (s0, srows) in stiles:
                            cf = cpool.tile([128, VIS], F32, tag="cf")
                            nc.sync.dma_start(
                                cf[0:srows, :],
                                ctx_dram[bitem, s0:s0 + srows, :])
                            cb = cpool.tile([128, VIS], BF16, tag="cb")
                            nc.vector.tensor_copy(cb[0:srows, :],
                                                  cf[0:srows, :])
                            tps = ctr.tile([128, KC_V, 128], BF16, tag="ctr")
                            for c in range(KC_V):
                                nc.tensor.transpose(
                                    tps[:, c, 0:srows],
                                    cb[0:srows, 128 * c:128 * (c + 1)],
                                    ident[0:srows, 0:srows])
                            nc.vector.tensor_copy(
                                ctxT[:, :, s0:s0 + srows], tps[:, :, 0:srows])
                        KTb = kv_pool.tile([64, H, S], BF16, tag="KTb")
                        for og in range(8):
                            kps = ktp.tile([64, 2, 256], F32, tag="kt")
                            for oi in range(2):
                                oc = 2 * og + oi
                                for kc in range(KC_V):
                                    nc.tensor.matmul(
                                        kps[0:64, oi, 0:S],
                                        wk[:, kc, 64 * oc:64 * (oc + 1)],
                                        ctxT[:, kc, :],
                                        start=(kc == 0),
                                        stop=(kc == KC_V - 1))
                            nc.vector.tensor_copy(
                                KTb[0:64, 2 * og:2 * (og + 1), :],
                                kps[0:64, :, 0:S])
                        Vb = kv_pool.tile([128, len(stiles), INNER], BF16,
                                          tag="Vb")
                        for si_, (s0, srows) in enumerate(stiles):
                            for nh in range(2):
                                vps = vpp.tile([128, 512], F32, tag="v")
                                for kc in range(KC_V):
                                    nc.tensor.matmul(
                                        vps[0:srows, :],
                                        ctxT[:, kc, s0:s0 + srows],
                                        wv[:, kc, 512 * nh:512 * (nh + 1)],
                                        start=(kc == 0),
                                        stop=(kc == KC_V - 1))
                                nc.vector.tensor_copy(
                                    Vb[0:srows, si_, 512 * nh:512 * (nh + 1)],
                                    vps[0:srows, :])

                        def v_lhsT(si_, h, _Vb=Vb, _stiles=stiles):
                            s0, srows = _stiles[si_]
                            return _Vb[0:srows, si_, 64 * h:64 * (h + 1)]

                        softmax_av(bitem, T * bitem, QT, KTb, S, stiles,
                                   v_lhsT, pools, attnT)
                with tc.tile_pool(name=pref + "wo", bufs=1) as wop:
                    wo = load_weight_bf16(wop, wdram[pref + "_wo"], INNER, D,
                                          "wo")
                    z_residual_stage(attnT, wo, bo, src_dram, dst_dram)

        # ---------------- phase 4: MLP ----------------
        def phase_mlp(src_dram, dst_dram):
            with contextlib.ExitStack() as ph:
                pc = ph.enter_context(tc.tile_pool(name="p4const", bufs=1))
                lnw = bcast_load(pc, wdram["ln4_w"], D, "lnw")
                lnb = bcast_load(pc, wdram["ln4_b"], D, "lnb")
                pjb = bcast_load(pc, wdram["proj_b"], D, "pjb")
                fcb = pc.tile([128, 32], F32, tag="fcb")
                nc.sync.dma_start(
                    fcb[:], wdram["fc_b"][:].rearrange("c p -> p c"))
                wpool = ph.enter_context(tc.tile_pool(name="p4w", bufs=1))
                fcw = wpool.tile([128, KC_D, 4 * D], BF16, tag="fcw")
                pjw = wpool.tile([128, 32, D], BF16, tag="pjw")
                with tc.tile_pool(name="p4stage", bufs=2) as stg:
                    fcv = wdram["fc_w"][:].rearrange("(c p) o -> p c o", p=128)
                    for r in range(16):
                        sf = stg.tile([128, KC_D, 256], F32, tag="ws")
                        nc.sync.dma_start(
                            sf[:], fcv[:, :, 256 * r:256 * (r + 1)])
                        nc.vector.tensor_copy(
                            fcw[:, :, 256 * r:256 * (r + 1)], sf[:])
                    pjv = wdram["proj_w"][:].rearrange("(c p) o -> p c o",
                                                       p=128)
                    for r in range(16):
                        sf2 = stg.tile([128, 32, 64], F32, tag="ws")
                        nc.sync.dma_start(
                            sf2[:], pjv[:, :, 64 * r:64 * (r + 1)])
                        nc.vector.tensor_copy(
                            pjw[:, :, 64 * r:64 * (r + 1)], sf2[:])
                with tc.tile_pool(name="p4hT", bufs=1) as hTp:
                    hT = hTp.tile([128, KC_D, NTOK], BF16, tag="hT")
                    ln_transpose_stage(src_dram, lnw, lnb, hT)
                    with contextlib.ExitStack() as gl:
                        up = gl.enter_context(tc.tile_pool(
                            name="p4u", bufs=2, space="PSUM"))
                        zp = gl.enter_context(tc.tile_pool(
                            name="p4z", bufs=1, space="PSUM"))
                        gp = gl.enter_context(tc.tile_pool(name="p4g",
                                                           bufs=1))
                        xo = gl.enter_context(tc.tile_pool(name="p4o",
                                                           bufs=1))
                        GS = 384
                        GC = 2.0 * np.sqrt(2.0 / np.pi)
                        for g in range(NTOK // GS):
                            z2 = zp.tile([128, 3, D], F32, tag="z2")
                            for fh in range(2):
                                uT = gp.tile([128, 16, GS], BF16, tag="uT")
                                for oi in range(16):
                                    oc = 16 * fh + oi
                                    ups = up.tile([128, GS], F32, tag="u")
                                    for kc in range(KC_D):
                                        nc.tensor.matmul(
                                            ups[:],
                                            fcw[:, kc,
                                                128 * oc:128 * (oc + 1)],
                                            hT[:, kc, GS * g:GS * (g + 1)],
                                            start=(kc == 0),
                                            stop=(kc == KC_D - 1))
                                    # gelu_tanh(ub) = ub * sigmoid(
                                    #   2c*(ub + 0.044715*ub^3)), exact match
                                    # to the reference's tanh approximation
                                    ub = gp.tile([128, GS], F32, tag="gub")
                                    nc.vector.tensor_scalar_add(
                                        ub[:], ups[:], fcb[:, oc:oc + 1])
                                    gs = gp.tile([128, GS], F32, tag="gs")
                                    nc.vector.tensor_mul(gs[:], ub[:], ub[:])
                                    nc.vector.tensor_scalar(
                                        out=gs[:], in0=gs[:],
                                        scalar1=0.044715, scalar2=1.0,
                                        op0=MUL, op1=ADD)
                                    nc.vector.tensor_mul(gs[:], gs[:], ub[:])
                                    sg = gp.tile([128, GS], F32, tag="gsg")
                                    nc.scalar.activation(
                                        sg[:], gs[:], AF.Sigmoid, scale=GC)
                                    nc.vector.tensor_mul(
                                        uT[:, oi, :], ub[:], sg[:])
                                for m in range(3):
                                    for nh in range(2):
                                        for oi in range(16):
                                            nc.tensor.matmul(
                                                z2[:, m,
                                                   512 * nh:512 * (nh + 1)],
                                                uT[:, oi,
                                                   128 * m:128 * (m + 1)],
                                                pjw[:, 16 * fh + oi,
                                                    512 * nh:512 * (nh + 1)],
                                                start=(fh == 0 and oi == 0),
                                                stop=(fh == 1 and oi == 15))
                            for m in range(3):
                                t_i = 3 * g + m
                                xt = xo.tile([128, D], F32, tag="xprev")
                                nc.sync.dma_start(
                                    xt[:],
                                    src_dram[128 * t_i:128 * (t_i + 1), :])
                                ot = xo.tile([128, D], F32, tag="xout")
                                nc.vector.tensor_add(ot[:], z2[:, m, :],
                                                     xt[:])
                                nc.vector.tensor_add(ot[:], ot[:], pjb[:])
                                nc.sync.dma_start(
                                    dst_dram[128 * t_i:128 * (t_i + 1), :],
                                    ot[:])

        dsts = [xs[0], xs[1], xs[2], out_d]

        def dst_for(i):
            return out_d if i == n_phases - 1 else dsts[i]

        try:
            if n_phases >= 1:
                phase_self(x_in, dst_for(0))
        except _StopBuild:
            return nc
        if n_phases >= 2:
            phase_cross("ia", img_in, SI, [(0, 128), (128, 69)],
                        dst_for(0), dst_for(1), 2)
        if n_phases >= 3:
            phase_cross("pa", pr_in, SP, [(0, 77)], dst_for(1), dst_for(2), 3)
        if n_phases >= 4:
            phase_mlp(dst_for(2), out_d)

    return nc


def make_in_map(inputs, core):
    """Slice full inputs for one core -> name->array map for the BIR."""
    b0 = core * B
    m = {
        "x": np.ascontiguousarray(
            inputs["x"][b0:b0 + B].reshape(NTOK, D), dtype=np.float32),
        "img_embd": np.ascontiguousarray(inputs["img_embd"][b0:b0 + B],
                                         dtype=np.float32),
        "prompt_embd": np.ascontiguousarray(inputs["prompt_embd"][b0:b0 + B],
                                            dtype=np.float32),
    }
    for k in ("sa", "ia", "pa"):
        for s in ("wq", "wk", "wv", "wo"):
            m[f"{k}_{s}"] = np.ascontiguousarray(inputs[f"{k}_{s}"],
                                                 dtype=np.float32)
        m[f"{k}_bo"] = np.ascontiguousarray(
            inputs[f"{k}_bo"].reshape(1, D), dtype=np.float32)
    for i in (1, 2, 3, 4):
        m[f"ln{i}_w"] = np.ascontiguousarray(
            inputs[f"ln{i}_w"].reshape(1, D), dtype=np.float32)
        m[f"ln{i}_b"] = np.ascontiguousarray(
            inputs[f"ln{i}_b"].reshape(1, D), dtype=np.float32)
    m["fc_w"] = np.ascontiguousarray(inputs["fc_w"], dtype=np.float32)
    m["fc_b"] = np.ascontiguousarray(
        inputs["fc_b"].reshape(32, 128), dtype=np.float32)
    m["proj_w"] = np.ascontiguousarray(inputs["proj_w"], dtype=np.float32)
    m["proj_b"] = np.ascontiguousarray(
        inputs["proj_b"].reshape(1, D), dtype=np.float32)
    return m


_CACHED = None


def _get_nc():
    global _CACHED
    if _CACHED is None:
        nc = build_nc()
        patch_nc_for_walrus(nc)
        _CACHED = nc
    return _CACHED


def kernel(**inputs):
    from concourse.bass_utils import run_bass_kernel_spmd

    nc = _get_nc()
    inputs = {k: np.asarray(v) for k, v in inputs.items()}
    in_maps = [make_in_map(inputs, c) for c in range(NCORES)]
    res = run_bass_kernel_spmd(nc, in_maps, core_ids=list(range(NCORES)))
    out = np.concatenate(
        [res.results[c]["out"].reshape(B, T, D) for c in range(NCORES)],
        axis=0)
    return out.astype(np.float32)


# revision 29
# speedup vs baseline: 1.4470x; 1.4470x over previous
"""Trainium2 Bass kernel for nn_Block_7584912244938.

Dense transformer block: self-attn + img cross-attn + prompt cross-attn + MLP,
d_model=1024, 16 heads x 64, batch 256 x 60 tokens.

Strategy: pure data parallel over batch across 8 NeuronCores (32 items/core).
Inside each core: token-major activations with PE transposes feeding
feature-major (transposed) matmul operands; bf16 matmuls with fp32 PSUM
accumulation; LN/softmax/residual arithmetic in fp32.

Self-contained: only imports from /opt/trn_rl_repo (baked into the container).
"""

import contextlib
import sys

sys.path.insert(0, "/opt/trn_rl_repo")

import ml_dtypes
import numpy as np
import orjson

import concourse.bass as bass
import concourse.tile as tile
from concourse import mybir

F32 = mybir.dt.float32
BF16 = mybir.dt.bfloat16
AX = mybir.AxisListType.X
MUL = mybir.AluOpType.mult
ADD = mybir.AluOpType.add
SUB = mybir.AluOpType.subtract
AF = mybir.ActivationFunctionType

# problem dims (per core)
NCORES = 8
B = 32            # items per core
T = 60            # x tokens per item
NTOK = B * T      # 1920
D = 1024
H = 16
DH = 64
INNER = 1024
VIS = 768
SI = 197          # img tokens per item
SP = 77           # prompt tokens per item
SCALE = DH ** -0.5
NT = NTOK // 128  # 15 token tiles of 128
KC_D = D // 128   # 8
KC_V = VIS // 128  # 6
EPS = 1e-5


def _fix_bir_waits(d):
    """This container's walrus accepts at most 1 sync-wait per instruction;
    Tile emits up to ~3. Split the excess onto inserted single-wait NoOps
    placed immediately before (same engine, so ordering is preserved)."""
    for fn in d["functions"]:
        for blk in fn["blocks"]:
            new_insts = []
            for ins in blk["instructions"]:
                si = ins.get("sync_info") or {}
                w = si.get("on_wait") or []
                cap = 1
                if len(w) > cap:
                    for j, ww in enumerate(w[:-cap]):
                        new_insts.append({
                            "engine": ins["engine"], "ins": [], "outs": [],
                            "is_reset_sema": False,
                            "name": f"{ins['name']}__ws{j}",
                            "opcode": "NoOp",
                            "sync_info": {"on_update": [], "on_wait": [ww]},
                        })
                    si = dict(si)
                    si["on_wait"] = w[-cap:]
                    ins = dict(ins)
                    ins["sync_info"] = si
                new_insts.append(ins)
            blk["instructions"] = new_insts


def patch_nc_for_walrus(nc):
    orig = nc.to_json_bytes

    def patched():
        d = orjson.loads(orig())
        _fix_bir_waits(d)
        return orjson.dumps(d)

    nc.to_json_bytes = patched


def build_nc(n_phases=4, stop_after=None):
    nc = bass.Bass("TRN2", target_bir_lowering=False, debug=False)

    x_in = nc.dram_tensor("x", [NTOK, D], F32, kind="ExternalInput")
    img_in = nc.dram_tensor("img_embd", [B, SI, VIS], F32, kind="ExternalInput")
    pr_in = nc.dram_tensor("prompt_embd", [B, SP, VIS], F32,
                           kind="ExternalInput")

    wdram = {}
    for p in ("sa", "ia", "pa"):
        ckin = D if p == "sa" else VIS
        wdram[p + "_wq"] = nc.dram_tensor(p + "_wq", [D, INNER], F32,
                                          kind="ExternalInput")
        wdram[p + "_wk"] = nc.dram_tensor(p + "_wk", [ckin, INNER], F32,
                                          kind="ExternalInput")
        wdram[p + "_wv"] = nc.dram_tensor(p + "_wv", [ckin, INNER], F32,
                                          kind="ExternalInput")
        wdram[p + "_wo"] = nc.dram_tensor(p + "_wo", [INNER, D], F32,
                                          kind="ExternalInput")
        wdram[p + "_bo"] = nc.dram_tensor(p + "_bo", [1, D], F32,
                                          kind="ExternalInput")
    for i in (1, 2, 3, 4):
        wdram[f"ln{i}_w"] = nc.dram_tensor(f"ln{i}_w", [1, D], F32,
                                           kind="ExternalInput")
        wdram[f"ln{i}_b"] = nc.dram_tensor(f"ln{i}_b", [1, D], F32,
                                           kind="ExternalInput")
    wdram["fc_w"] = nc.dram_tensor("fc_w", [D, 4 * D], F32,
                                   kind="ExternalInput")
    wdram["fc_b"] = nc.dram_tensor("fc_b", [32, 128], F32,
                                   kind="ExternalInput")
    wdram["proj_w"] = nc.dram_tensor("proj_w", [4 * D, D], F32,
                                     kind="ExternalInput")
    wdram["proj_b"] = nc.dram_tensor("proj_b", [1, D], F32,
                                     kind="ExternalInput")

    out_d = nc.dram_tensor("out", [NTOK, D], F32, kind="ExternalOutput")
    xs = [nc.dram_tensor(f"x_scr{i}", [NTOK, D], F32) for i in range(3)]
    ident_bd = nc.inline_tensor(np.eye(128).astype(ml_dtypes.bfloat16),
                                name="ident_bf")
    ones_d = nc.inline_tensor(np.ones((1, 128), np.float32), name="ones_f32")

    with tile.TileContext(nc) as tc, contextlib.ExitStack() as top:
        const = top.enter_context(tc.tile_pool(name="const", bufs=1))
        ident = const.tile([128, 128], BF16, tag="ident")
        nc.sync.dma_start(ident[:], ident_bd[:])
        ones_f = const.tile([1, 128], F32, tag="ones")
        nc.sync.dma_start(ones_f[:], ones_d[:])

        def bcast_load(pool, dram_h, cols, tag):
            # broadcast [1, cols] across partitions via a K=1 matmul with a
            # ones column (this walrus rejects stride-0 partition DMA APs)
            row = pool.tile([1, cols], F32, tag=tag + "r")
            nc.sync.dma_start(row[:], dram_h[:])
            t = pool.tile([128, cols], F32, tag=tag)
            with tc.tile_pool(name="bc_" + tag, bufs=2, space="PSUM") as bp:
                for j in range(cols // 512):
                    ps = bp.tile([128, 512], F32, tag="b")
                    nc.tensor.matmul(ps[:], ones_f[:, :],
                                     row[:, 512 * j:512 * (j + 1)],
                                     start=True, stop=True)
                    nc.vector.tensor_copy(t[:, 512 * j:512 * (j + 1)], ps[:])
            return t

        def load_weight_bf16(dst_pool, dram_h, kin, kout, tag):
            """[kin,kout] f32 dram -> [128, kin//128, kout] bf16 sbuf."""
            kc = kin // 128
            with tc.tile_pool(name="wstage_" + tag, bufs=1) as sp:
                wf = sp.tile([128, kc, kout], F32, tag="wstage")
                nc.sync.dma_start(
                    wf[:], dram_h[:].rearrange("(c p) o -> p c o", p=128))
                wb = dst_pool.tile([128, kc, kout], BF16, tag=tag)
                nc.vector.tensor_copy(wb[:], wf[:])
            return wb

        def ln_transpose_stage(src_dram, lnw_t, lnb_t, hT):
            """LayerNorm all 15 x-tiles of src; write transposed bf16 into
            hT [128, 8, 1920]."""
            with contextlib.ExitStack() as st:
                xp = st.enter_context(tc.tile_pool(name="ln_x", bufs=2))
                stat = st.enter_context(tc.tile_pool(name="ln_stat", bufs=4))
                tp = st.enter_context(
                    tc.tile_pool(name="ln_tr", bufs=2, space="PSUM"))
                for t_i in range(NT):
                    xt = xp.tile([128, D], F32, tag="x")
                    nc.sync.dma_start(
                        xt[:], src_dram[128 * t_i:128 * (t_i + 1), :])
                    s1 = stat.tile([128, 1], F32, tag="s1")
                    nc.vector.reduce_sum(s1[:], xt[:], axis=AX)
                    sq = xp.tile([128, D], F32, tag="scr")
                    nc.vector.tensor_mul(sq[:], xt[:], xt[:])
                    ssq = stat.tile([128, 1], F32, tag="ssq")
                    nc.vector.reduce_sum(ssq[:], sq[:], axis=AX)
                    mu = stat.tile([128, 1], F32, tag="mu")
                    nc.vector.tensor_scalar_mul(mu[:], s1[:], 1.0 / D)
                    mu2 = stat.tile([128, 1], F32, tag="mu2")
                    nc.vector.tensor_mul(mu2[:], mu[:], mu[:])
                    var = stat.tile([128, 1], F32, tag="var")
                    nc.vector.tensor_scalar_mul(var[:], ssq[:], 1.0 / D)
                    nc.vector.tensor_sub(var[:], var[:], mu2[:])
                    nc.vector.tensor_scalar_add(var[:], var[:], EPS)
                    # this walrus requires ACT ops to be [128, >=128]: run the
                    # sqrt on a zero-padded [128, 128] tile
                    varp = stat.tile([128, 128], F32, tag="varp")
                    nc.vector.memset(varp[:], 1.0)
                    nc.vector.tensor_copy(varp[:, 0:1], var[:])
                    sdp = stat.tile([128, 128], F32, tag="sdp")
                    nc.scalar.activation(sdp[:], varp[:], AF.Sqrt)
                    rstd = stat.tile([128, 1], F32, tag="rstd")
                    nc.vector.reciprocal(rstd[:], sdp[:, 0:1])
                    xn = xp.tile([128, D], F32, tag="scr")
                    nc.vector.tensor_scalar(
                        out=xn[:], in0=xt[:], scalar1=mu[:], scalar2=rstd[:],
                        op0=SUB, op1=MUL)
                    nc.vector.tensor_mul(xn[:], xn[:], lnw_t[:])
                    hb = xp.tile([128, D], BF16, tag="hb")
                    nc.vector.tensor_add(hb[:], xn[:], lnb_t[:])
                    ps = tp.tile([128, KC_D, 128], BF16, tag="tr")
                    for c in range(KC_D):
                        nc.tensor.transpose(
                            ps[:, c, :], hb[:, 128 * c:128 * (c + 1)],
                            ident[:])
                    nc.vector.tensor_copy(
                        hT[:, :, 128 * t_i:128 * (t_i + 1)], ps[:])

        def project_T(w_bf, src_T, kc_n, dst_T, gsz=480):
            """dst_T [128, 8, NTOK] bf16 (feature-major) = w.T @ src_T."""
            with tc.tile_pool(name="projT", bufs=2, space="PSUM") as pp:
                for g in range(NTOK // gsz):
                    for oc in range(KC_D):
                        ps = pp.tile([128, gsz], F32, tag="p")
                        for kc in range(kc_n):
                            nc.tensor.matmul(
                                ps[:],
                                w_bf[:, kc, 128 * oc:128 * (oc + 1)],
                                src_T[:, kc, gsz * g:gsz * (g + 1)],
                                start=(kc == 0), stop=(kc == kc_n - 1))
                        nc.vector.tensor_copy(
                            dst_T[:, oc, gsz * g:gsz * (g + 1)], ps[:])

        def project_T_pad(w_bf, src_T, kc_n, dst_T, src0, ncols,
                          gsz=480):
            """Head-padded projection: dst_T[0:64, 16, ncols] bf16 =
            per-64-wide-output-slice w.T @ src_T (M=64, base 0)."""
            with tc.tile_pool(name="projP", bufs=2, space="PSUM") as pp:
                for g in range(ncols // gsz):
                    for oc in range(16):
                        ps = pp.tile([64, gsz], F32, tag="p")
                        for kc in range(kc_n):
                            nc.tensor.matmul(
                                ps[:],
                                w_bf[:, kc, 64 * oc:64 * (oc + 1)],
                                src_T[:, kc,
                                      src0 + gsz * g:src0 + gsz * (g + 1)],
                                start=(kc == 0), stop=(kc == kc_n - 1))
                        nc.vector.tensor_copy(
                            dst_T[0:64, oc, gsz * g:gsz * (g + 1)], ps[:])

        def softmax_av(bitem, lb, QT, KTb, S, stiles, v_lhsT, pools,
                       attnT):
            """Scores + exp-softmax + P-transposes + AV for one item.
            QT/KTb are head-padded: [rows 0:64, 16 heads, cols] bf16, so
            every matmul contracts K=64 at partition base 0 (this stack
            hangs on stationary operands at base 64). lb = local col base
            (token col of this item within QT). Scores use M=128 (reads
            zero-padded neighbor cols) so exp sees 128 partitions."""
            simp, pp_pt, pp_av, p_pool, pt_pool, stat = pools
            W = 256 if S > 64 else 64
            P = p_pool.tile([128, H, S], BF16, tag="P")
            for q in range(4):
                sps = simp.tile([128, 4, W], F32, tag="sim")
                for hq in range(4):
                    h = 4 * q + hq
                    nc.tensor.matmul(
                        sps[:, hq, 0:S],
                        QT[0:64, h, lb:lb + 128],
                        KTb[0:64, h, 0:S],
                        start=True, stop=True)
                sc = p_pool.tile([128, 4, W], F32, tag="sc")
                nc.vector.tensor_copy(sc[:, :, 0:S], sps[:, :, 0:S])
                nc.scalar.activation(
                    P[:, 4 * q:4 * (q + 1), :], sc[:, :, 0:S], AF.Exp,
                    scale=SCALE)
            rs = stat.tile([128, H, 1], F32, tag="rs")
            nc.vector.reduce_sum(rs[:, :, 0:1], P[:], axis=AX)
            rsi = stat.tile([128, H, 1], F32, tag="rsi")
            nc.vector.reciprocal(rsi[:], rs[:])
            rsb = stat.tile([128, H, 1], BF16, tag="rsb")
            nc.vector.tensor_copy(rsb[:], rsi[:])
            nc.vector.tensor_tensor(
                out=P[:], in0=P[:], in1=rsb[:].to_broadcast((128, H, S)),
                op=MUL)
            nblk = H * len(stiles)
            PTs = pt_pool.tile([128, nblk, 64], BF16, tag="PTs")
            for si_, (s0, srows) in enumerate(stiles):
                pt_ps = pp_pt.tile([128, H, 64], BF16, tag="pt")
                for h in range(H):
                    nc.tensor.transpose(
                        pt_ps[0:srows, h, 0:60], P[0:60, h, s0:s0 + srows],
                        ident[0:60, 0:60])
                nc.vector.tensor_copy(
                    PTs[0:srows, si_ * H:(si_ + 1) * H, 0:60],
                    pt_ps[0:srows, :, 0:60])
            avp = pp_av.tile([128, KC_D, 64], F32, tag="av")
            for h in range(H):
                rb = (h % 2) * 64
                for si_, (s0, srows) in enumerate(stiles):
                    nc.tensor.matmul(
                        avp[rb:rb + 64, h // 2, 0:60],
                        v_lhsT(si_, h),
                        PTs[0:srows, si_ * H + h, 0:60],
                        start=(si_ == 0), stop=(si_ == len(stiles) - 1))
            nc.vector.tensor_copy(
                attnT[:, :, T * bitem:T * bitem + T], avp[:, :, 0:60])

        def z_residual_stage(attnT, wo_bf, bo_t, xprev_dram, xnext_dram):
            with contextlib.ExitStack() as st:
                zp = st.enter_context(
                    tc.tile_pool(name="zps", bufs=2, space="PSUM"))
                xo = st.enter_context(tc.tile_pool(name="zout", bufs=3))
                for g in range(NT):
                    xt = xo.tile([128, D], F32, tag="xprev")
                    nc.sync.dma_start(
                        xt[:], xprev_dram[128 * g:128 * (g + 1), :])
                    ot = xo.tile([128, D], F32, tag="xout")
                    for nh in range(2):
                        ps = zp.tile([128, 512], F32, tag="z")
                        for kc in range(KC_D):
                            nc.tensor.matmul(
                                ps[:],
                                attnT[:, kc, 128 * g:128 * (g + 1)],
                                wo_bf[:, kc, 512 * nh:512 * (nh + 1)],
                                start=(kc == 0), stop=(kc == KC_D - 1))
                        nc.vector.tensor_add(
                            ot[:, 512 * nh:512 * (nh + 1)], ps[:],
                            xt[:, 512 * nh:512 * (nh + 1)])
                    nc.vector.tensor_add(ot[:], ot[:], bo_t[:])
                    nc.sync.dma_start(
                        xnext_dram[128 * g:128 * (g + 1), :], ot[:])

        # ---------------- phase 1: self attention ----------------
        class _StopBuild(Exception):
            pass

        def _maybe_stop(stage):
            if stop_after == stage:
                raise _StopBuild()

        def phase_self(src_dram, dst_dram):
            with contextlib.ExitStack() as ph:
                pc = ph.enter_context(tc.tile_pool(name="p1const", bufs=1))
                lnw = bcast_load(pc, wdram["ln1_w"], D, "lnw")
                lnb = bcast_load(pc, wdram["ln1_b"], D, "lnb")
                bo = bcast_load(pc, wdram["sa_bo"], D, "bo")
                big = ph.enter_context(tc.tile_pool(name="p1big", bufs=1))
                attnT = big.tile([128, KC_D, NTOK], BF16, tag="attnT")

                with tc.tile_pool(name="p1hT", bufs=1) as hTp:
                    hT = hTp.tile([128, KC_D, NTOK], BF16, tag="hT")
                    _maybe_stop("const")
                    ln_transpose_stage(src_dram, lnw, lnb, hT)
                    _maybe_stop("ln")
                    with tc.tile_pool(name="p1w", bufs=1) as wp, \
                         tc.tile_pool(name="p1qk", bufs=1) as qk:
                        wq = load_weight_bf16(wp, wdram["sa_wq"], D, INNER,
                                              "wq")
                        wk = load_weight_bf16(wp, wdram["sa_wk"], D, INNER,
                                              "wk")
                        wv = load_weight_bf16(wp, wdram["sa_wv"], D, INNER,
                                              "wv")
                        HC = 480 + 128
                        for half in range(4):
                            QTh = qk.tile([64, H, HC], BF16, tag="QTh")
                            for h in range(H):
                                nc.vector.memset(QTh[0:64, h, 480:HC], 0.0)
                            project_T_pad(wq, hT, KC_D, QTh, 480 * half, 480)
                            KTh = qk.tile([64, H, 480], BF16, tag="KTh")
                            project_T_pad(wk, hT, KC_D, KTh, 480 * half, 480)
                            _maybe_stop("qt" if half == 0 else "qt2")
                            with contextlib.ExitStack() as hh:
                                vpool = hh.enter_context(
                                    tc.tile_pool(name="p1V", bufs=1))
                                Vh = vpool.tile([60, 8, INNER], BF16,
                                                tag="V")
                                vps_p = hh.enter_context(tc.tile_pool(
                                    name="p1vps", bufs=2, space="PSUM"))
                                items = list(range(8 * half,
                                                   8 * (half + 1)))
                                for li, bitem in enumerate(items):
                                    for nh in range(2):
                                        vps = vps_p.tile([60, 512], F32,
                                                         tag="vps")
                                        for kc in range(KC_D):
                                            nc.tensor.matmul(
                                                vps[:],
                                                hT[:, kc,
                                                   T * bitem:T * bitem + T],
                                                wv[:, kc,
                                                   512 * nh:512 * (nh + 1)],
                                                start=(kc == 0),
                                                stop=(kc == KC_D - 1))
                                        nc.vector.tensor_copy(
                                            Vh[:, li,
                                               512 * nh:512 * (nh + 1)],
                                            vps[:])
                                _maybe_stop("v" if half == 0 else "v2")
                                simp = hh.enter_context(tc.tile_pool(
                                    name="p1sim", bufs=2, space="PSUM"))
                                pp_pt = hh.enter_context(tc.tile_pool(
                                    name="p1pt", bufs=2, space="PSUM"))
                                pp_av = hh.enter_context(tc.tile_pool(
                                    name="p1av", bufs=2, space="PSUM"))
                                p_pool = hh.enter_context(
                                    tc.tile_pool(name="p1P", bufs=2))
                                pt_pool = hh.enter_context(
                                    tc.tile_pool(name="p1PT", bufs=2))
                                stat = hh.enter_context(
                                    tc.tile_pool(name="p1st", bufs=2))
                                pools = (simp, pp_pt, pp_av, p_pool, pt_pool,
                                         stat)
                                for li, bitem in enumerate(items):
                                    lt = T * li

                                    def v_lhsT(si_, h, _li=li):
                                        return Vh[0:T, _li,
                                                  64 * h:64 * (h + 1)]

                                    softmax_av(
                                        bitem, lt, QTh,
                                        KTh[:, :, lt:lt + T], T, [(0, T)],
                                        v_lhsT, pools, attnT)
                _maybe_stop("attn")
                with tc.tile_pool(name="p1wo", bufs=1) as wop:
                    wo = load_weight_bf16(wop, wdram["sa_wo"], INNER, D,
                                          "wo")
                    z_residual_stage(attnT, wo, bo, src_dram, dst_dram)

        # ---------------- phases 2/3: cross attention ----------------
        def phase_cross(pref, ctx_dram, S, stiles, src_dram, dst_dram, ln_i):
            with contextlib.ExitStack() as ph:
                pc = ph.enter_context(tc.tile_pool(name=pref + "const",
                                                   bufs=1))
                lnw = bcast_load(pc, wdram[f"ln{ln_i}_w"], D, "lnw")
                lnb = bcast_load(pc, wdram[f"ln{ln_i}_b"], D, "lnb")
                bo = bcast_load(pc, wdram[pref + "_bo"], D, "bo")
                big = ph.enter_context(tc.tile_pool(name=pref + "big",
                                                    bufs=1))
                attnT = big.tile([128, KC_D, NTOK], BF16, tag="attnT")
                QT = big.tile([64, H, NTOK + 128], BF16, tag="QT")
                for h in range(H):
                    nc.vector.memset(QT[0:64, h, NTOK:NTOK + 128], 0.0)
                with tc.tile_pool(name=pref + "hT", bufs=1) as hTp:
                    hT = hTp.tile([128, KC_D, NTOK], BF16, tag="hT")
                    ln_transpose_stage(src_dram, lnw, lnb, hT)
                    with tc.tile_pool(name=pref + "wq", bufs=1) as wp:
                        wq = load_weight_bf16(wp, wdram[pref + "_wq"], D,
                                              INNER, "wq")
                        project_T_pad(wq, hT, KC_D, QT, 0, NTOK)
                with contextlib.ExitStack() as il:
                    wkp = il.enter_context(tc.tile_pool(name=pref + "wk",
                                                        bufs=1))
                    wk = load_weight_bf16(wkp, wdram[pref + "_wk"], VIS,
                                          INNER, "wk")
                    wv = load_weight_bf16(wkp, wdram[pref + "_wv"], VIS,
                                          INNER, "wv")
                    cpool = il.enter_context(tc.tile_pool(name=pref + "c",
                                                          bufs=2))
                    ctr = il.enter_context(tc.tile_pool(
                        name=pref + "ctr", bufs=1, space="PSUM"))
                    ktp = il.enter_context(tc.tile_pool(
                        name=pref + "ktp", bufs=1, space="PSUM"))
                    vpp = il.enter_context(tc.tile_pool(
                        name=pref + "vps", bufs=1, space="PSUM"))
                    simp = il.enter_context(tc.tile_pool(
                        name=pref + "sim", bufs=1, space="PSUM"))
                    pp_pt = il.enter_context(tc.tile_pool(
                        name=pref + "pt", bufs=1, space="PSUM"))
                    pp_av = il.enter_context(tc.tile_pool(
                        name=pref + "av", bufs=2, space="PSUM"))
                    kv_pool = il.enter_context(tc.tile_pool(
                        name=pref + "kv", bufs=2))
                    p_pool = il.enter_context(tc.tile_pool(
                        name=pref + "P", bufs=2))
                    pt_pool = il.enter_context(tc.tile_pool(
                        name=pref + "PT", bufs=2))
                    stat = il.enter_context(tc.tile_pool(
                        name=pref + "st", bufs=2))
                    pools = (simp, pp_pt, pp_av, p_pool, pt_pool, stat)
                    for bitem in range(B):
                        ctxT = kv_pool.tile([128, KC_V, S], BF16, tag="ctxT")
                        for (s0, srows) in stiles:
                            cf = cpool.tile([128, VIS], F32, tag="cf")
                            nc.sync.dma_start(
                                cf[0:srows, :],
                                ctx_dram[bitem, s0:s0 + srows, :])
                            cb = cpool.tile([128, VIS], BF16, tag="cb")
                            nc.vector.tensor_copy(cb[0:srows, :],
                                                  cf[0:srows, :])
                            tps = ctr.tile([128, KC_V, 128], BF16, tag="ctr")
                            for c in range(KC_V):
                                nc.tensor.transpose(
                                    tps[:, c, 0:srows],
                                    cb[0:srows, 128 * c:128 * (c + 1)],
                                    ident[0:srows, 0:srows])
                            nc.vector.tensor_copy(
                                ctxT[:, :, s0:s0 + srows], tps[:, :, 0:srows])
                        KTb = kv_pool.tile([64, H, S], BF16, tag="KTb")
                        for og in range(8):
                            kps = ktp.tile([64, 2, 256], F32, tag="kt")
                            for oi in range(2):
                                oc = 2 * og + oi
                                for kc in range(KC_V):
                                    nc.tensor.matmul(
                                        kps[0:64, oi, 0:S],
                                        wk[:, kc, 64 * oc:64 * (oc + 1)],
                                        ctxT[:, kc, :],
                                        start=(kc == 0),
                                        stop=(kc == KC_V - 1))
                            nc.vector.tensor_copy(
                                KTb[0:64, 2 * og:2 * (og + 1), :],
                                kps[0:64, :, 0:S])
                        Vb = kv_pool.tile([128, len(stiles), INNER], BF16,
                                          tag="Vb")
                        for si_, (s0, srows) in enumerate(stiles):
                            for nh in range(2):
                                vps = vpp.tile([128, 512], F32, tag="v")
                                for kc in range(KC_V):
                                    nc.tensor.matmul(
                                        vps[0:srows, :],
                                        ctxT[:, kc, s0:s0 + srows],
                                        wv[:, kc, 512 * nh:512 * (nh + 1)],
                                        start=(kc == 0),
                                        stop=(kc == KC_V - 1))
                                nc.vector.tensor_copy(
                                    Vb[0:srows, si_, 512 * nh:512 * (nh + 1)],
                                    vps[0:srows, :])

                        def v_lhsT(si_, h, _Vb=Vb, _stiles=stiles):
                            s0, srows = _stiles[si_]
                            return _Vb[0:srows, si_, 64 * h:64 * (h + 1)]

                        softmax_av(bitem, T * bitem, QT, KTb, S, stiles,
                                   v_lhsT, pools, attnT)
                with tc.tile_pool(name=pref + "wo", bufs=1) as wop:
                    wo = load_weight_bf16(wop, wdram[pref + "_wo"], INNER, D,
                                          "wo")
                    z_residual_stage(attnT, wo, bo, src_dram, dst_dram)

        # ---------------- phase 4: MLP ----------------
        def phase_mlp(src_dram, dst_dram):
            with contextlib.ExitStack() as ph:
                pc = ph.enter_context(tc.tile_pool(name="p4const", bufs=1))
                lnw = bcast_load(pc, wdram["ln4_w"], D, "lnw")
                lnb = bcast_load(pc, wdram["ln4_b"], D, "lnb")
                pjb = bcast_load(pc, wdram["proj_b"], D, "pjb")
                fcb = pc.tile([128, 32], F32, tag="fcb")
                nc.sync.dma_start(
                    fcb[:], wdram["fc_b"][:].rearrange("c p -> p c"))
                wpool = ph.enter_context(tc.tile_pool(name="p4w", bufs=1))
                fcw = wpool.tile([128, KC_D, 4 * D], BF16, tag="fcw")
                pjw = wpool.tile([128, 32, D], BF16, tag="pjw")
                with tc.tile_pool(name="p4stage", bufs=2) as stg:
                    fcv = wdram["fc_w"][:].rearrange("(c p) o -> p c o", p=128)
                    for r in range(16):
                        sf = stg.tile([128, KC_D, 256], F32, tag="ws")
                        nc.sync.dma_start(
                            sf[:], fcv[:, :, 256 * r:256 * (r + 1)])
                        nc.vector.tensor_copy(
                            fcw[:, :, 256 * r:256 * (r + 1)], sf[:])
                    pjv = wdram["proj_w"][:].rearrange("(c p) o -> p c o",
                                                       p=128)
                    for r in range(16):
                        sf2 = stg.tile([128, 32, 64], F32, tag="ws")
                        nc.sync.dma_start(
                            sf2[:], pjv[:, :, 64 * r:64 * (r + 1)])
                        nc.vector.tensor_copy(
                            pjw[:, :, 64 * r:64 * (r + 1)], sf2[:])
                with tc.tile_pool(name="p4hT", bufs=1) as hTp:
                    hT = hTp.tile([128, KC_D, NTOK], BF16, tag="hT")
                    ln_transpose_stage(src_dram, lnw, lnb, hT)
                    with contextlib.ExitStack() as gl:
                        up = gl.enter_context(tc.tile_pool(
                            name="p4u", bufs=2, space="PSUM"))
                        zp = gl.enter_context(tc.tile_pool(
                            name="p4z", bufs=1, space="PSUM"))
                        gp = gl.enter_context(tc.tile_pool(name="p4g",
                                                           bufs=1))
                        xo = gl.enter_context(tc.tile_pool(name="p4o",
                                                           bufs=1))
                        GS = 384
                        GC = 2.0 * np.sqrt(2.0 / np.pi)
                        for g in range(NTOK // GS):
                            z2 = zp.tile([128, 3, D], F32, tag="z2")
                            for fh in range(2):
                                uT = gp.tile([128, 16, GS], BF16, tag="uT")
                                for oi in range(16):
                                    oc = 16 * fh + oi
                                    ups = up.tile([128, GS], F32, tag="u")
                                    for kc in range(KC_D):
                                        nc.tensor.matmul(
                                            ups[:],
                                            fcw[:, kc,
                                                128 * oc:128 * (oc + 1)],
                                            hT[:, kc, GS * g:GS * (g + 1)],
                                            start=(kc == 0),
                                            stop=(kc == KC_D - 1))
                                    # gelu_tanh(ub) = ub * sigmoid(
                                    #   2c*(ub + 0.044715*ub^3)), exact match
                                    # to the reference's tanh approximation
                                    ub = gp.tile([128, GS], F32, tag="gub")
                                    nc.vector.tensor_scalar_add(
                                        ub[:], ups[:], fcb[:, oc:oc + 1])
                                    gs = gp.tile([128, GS], F32, tag="gs")
                                    nc.vector.tensor_mul(gs[:], ub[:], ub[:])
                                    nc.vector.tensor_scalar(
                                        out=gs[:], in0=gs[:],
                                        scalar1=0.044715, scalar2=1.0,
                                        op0=MUL, op1=ADD)
                                    nc.vector.tensor_mul(gs[:], gs[:], ub[:])
                                    sg = gp.tile([128, GS], F32, tag="gsg")
                                    nc.scalar.activation(
                                        sg[:], gs[:], AF.Sigmoid, scale=GC)
                                    nc.vector.tensor_mul(
                                        uT[:, oi, :], ub[:], sg[:])
                                for m in range(3):
                                    for nh in range(2):
                                        for oi in range(16):
                                            nc.tensor.matmul(
                                                z2[:, m,
                                                   512 * nh:512 * (nh + 1)],
                                                uT[:, oi,
                                                   128 * m:128 * (m + 1)],
                                                pjw[:, 16 * fh + oi,
                                                    512 * nh:512 * (nh + 1)],
                                                start=(fh == 0 and oi == 0),
                                                stop=(fh == 1 and oi == 15))
                            for m in range(3):
                                t_i = 3 * g + m
                                xt = xo.tile([128, D], F32, tag="xprev")
                                nc.sync.dma_start(
                                    xt[:],
                                    src_dram[128 * t_i:128 * (t_i + 1), :])
                                ot = xo.tile([128, D], F32, tag="xout")
                                nc.vector.tensor_add(ot[:], z2[:, m, :],
                                                     xt[:])
                                nc.vector.tensor_add(ot[:], ot[:], pjb[:])
                                nc.sync.dma_start(
                                    dst_dram[128 * t_i:128 * (t_i + 1), :],
                                    ot[:])

        dsts = [xs[0], xs[1], xs[2], out_d]

        def dst_for(i):
            return out_d if i == n_phases - 1 else dsts[i]

        try:
            if n_phases >= 1:
                phase_self(x_in, dst_for(0))
        except _StopBuild:
            return nc
        if n_phases >= 2:
            phase_cross("ia", img_in, SI, [(0, 128), (128, 69)],
                        dst_for(0), dst_for(1), 2)
        if n_phases >= 3:
            phase_cross("pa", pr_in, SP, [(0, 77)], dst_for(1), dst_for(2), 3)
        if n_phases >= 4:
            phase_mlp(dst_for(2), out_d)

    return nc


def make_in_map(inputs, core):
    """Slice full inputs for one core -> name->array map for the BIR."""
    b0 = core * B
    m = {
        "x": np.ascontiguousarray(
            inputs["x"][b0:b0 + B].reshape(NTOK, D), dtype=np.float32),
        "img_embd": np.ascontiguousarray(inputs["img_embd"][b0:b0 + B],
                                         dtype=np.float32),
        "prompt_embd": np.ascontiguousarray(inputs["prompt_embd"][b0:b0 + B],
                                            dtype=np.float32),
    }
    for k in ("sa", "ia", "pa"):
        for s in ("wq", "wk", "wv", "wo"):
            m[f"{k}_{s}"] = np.ascontiguousarray(inputs[f"{k}_{s}"],
                                                 dtype=np.float32)
        m[f"{k}_bo"] = np.ascontiguousarray(
            inputs[f"{k}_bo"].reshape(1, D), dtype=np.float32)
    for i in (1, 2, 3, 4):
        m[f"ln{i}_w"] = np.ascontiguousarray(
            inputs[f"ln{i}_w"].reshape(1, D), dtype=np.float32)
        m[f"ln{i}_b"] = np.ascontiguousarray(
            inputs[f"ln{i}_b"].reshape(1, D), dtype=np.float32)
    m["fc_w"] = np.ascontiguousarray(inputs["fc_w"], dtype=np.float32)
    m["fc_b"] = np.ascontiguousarray(
        inputs["fc_b"].reshape(32, 128), dtype=np.float32)
    m["proj_w"] = np.ascontiguousarray(inputs["proj_w"], dtype=np.float32)
    m["proj_b"] = np.ascontiguousarray(
        inputs["proj_b"].reshape(1, D), dtype=np.float32)
    return m


_CACHED = None


def _get_nc():
    global _CACHED
    if _CACHED is None:
        nc = build_nc()
        patch_nc_for_walrus(nc)
        _CACHED = nc
    return _CACHED


def kernel(**inputs):
    from concourse.bass_utils import run_bass_kernel_spmd

    nc = _get_nc()
    inputs = {k: np.asarray(v) for k, v in inputs.items()}
    in_maps = [make_in_map(inputs, c) for c in range(NCORES)]
    res = run_bass_kernel_spmd(nc, in_maps, core_ids=list(range(NCORES)))
    out = np.concatenate(
        [res.results[c]["out"].reshape(B, T, D) for c in range(NCORES)],
        axis=0)
    return out.astype(np.float32)
